# revision 32
# baseline (speedup 1.0000x reference)
"""DeepseekV2 MLA attention forward — Trainium2 Bass kernel (8 NeuronCores).

Sharding: data-parallel over batch (2) x sequence-parallel over query rows
(4 panels of 512) = 8 cores, no cross-core communication.  Each core:
  - q path (q_a_proj -> rmsnorm -> q_b_proj) for its 512 query rows
  - kv path (kv_a_proj -> rmsnorm -> kv_b_proj) for the FULL key sequence
  - RoPE, full attention (16 heads), o_proj for its query rows

Schedule notes (v2):
  - every matmul is bf16 x bf16 with fp32 PSUM accumulation (weights are
    pre-tiled contiguously on the host; activations are written back from
    PSUM as bf16).  FWL halves LDWEIGHTS; DMA and SBUF traffic halve.
  - qa / oT live in SBUF across phases (no DRAM round trip).
  - softmax denominators accumulate on DVE (no PE reduce per key block) into
    a [16, W] table; one batched reciprocal for heads 0..14 runs under the
    last group's attention, so only head 15's reciprocal is exposed before
    o_proj starts.  The numerator and denominator consume the same rounded
    bf16 probs, so the bf16 error largely cancels in the ratio.
  - q_pe lives on partitions 0:63 for all 16 heads so every rope-score
    matmul of a key block shares the same k_pe stationary tile.
"""

import os
import numpy as np
from contextlib import ExitStack

import ml_dtypes

import concourse.bass as bass
import concourse.bacc as bacc
import concourse.mybir as mybir
import concourse.tile as tile
from concourse import bass_utils

B, S, HID = 2, 2048, 2048
NH = 16
QLR, KVLR = 1536, 512
DN, DR, DV = 128, 64, 128
DQK = DN + DR
SCALE = DQK ** -0.5
EPS = 1e-6
P = 128
NPANEL = 4
W = S // NPANEL            # 512 query rows per core
NCORES = B * NPANEL

F32 = mybir.dt.float32
F32R = mybir.dt.float32r
BF16 = mybir.dt.bfloat16
EXP = mybir.ActivationFunctionType.Exp
SQRT = mybir.ActivationFunctionType.Sqrt
COPY = mybir.ActivationFunctionType.Copy
MULT = mybir.AluOpType.mult
ADD = mybir.AluOpType.add

KB_HID = HID // P          # 16
KB_QLR = QLR // P          # 12
KB_CKV = KVLR // P         # 4
KB_S = S // P              # 16
MB_QLR = QLR // P          # 12
MB_NOPE = NH * DN // P     # 16
MB_PE = NH * DR // P       # 8
MB_HID = HID // P          # 16
MB_KVA = 5                 # w_kva padded 576 -> 640 columns
NCH = S // W               # 4 column chunks of the full sequence

LAST_RESULT = None


def _mm(nc, out, lhsT, rhs, start, stop):
    l = lhsT if lhsT.dtype == BF16 else lhsT.bitcast(F32R)
    r = rhs if rhs.dtype == BF16 else rhs.bitcast(F32R)
    nc.tensor.matmul(out, l, r, start=start, stop=stop)


def _emit(tc, t, with_mask):
    nc = tc.nc

    with ExitStack() as big:
        const = big.enter_context(tc.tile_pool(name="const", bufs=1))
        ones_cf = const.tile([P, 1], F32)
        nc.vector.memset(ones_cf[:], 1.0)
        ones_rf = const.tile([1, P], F32)
        nc.vector.memset(ones_rf[:], 1.0)
        ones_col = const.tile([P, 1], F32R)
        nc.scalar.activation(ones_col[:], ones_cf[:], COPY)
        ones_row = const.tile([1, P], F32R)
        nc.scalar.activation(ones_row[:], ones_rf[:], COPY)
        eps1 = const.tile([1, 1], F32)
        nc.vector.memset(eps1[:], EPS)

        # persistent activation tiles (live into phase D/E)
        persist = big.enter_context(tc.tile_pool(name="persist", bufs=1))
        ckT = persist.tile([P, KB_CKV, S], BF16)       # 16 KB/part
        kpe64 = persist.tile([DR, S], BF16)            # k_pe, roped
        qnopeT = persist.tile([P, MB_NOPE, W], BF16)   # 16 KB/part
        qpe64 = persist.tile([DR, NH, W], BF16)        # q_pe, all heads, 0:63
        oT = persist.tile([P, NH, W], BF16)            # 16 KB/part
        prallA = persist.tile([NH - 2, W], F32)        # denominators h 0..13
        prrA = persist.tile([NH - 2, W], BF16)
        prallB = persist.tile([2, W], F32)             # denominators h 14,15
        prrB = persist.tile([2, W], BF16)
        selA = persist.tile([NH - 2, NH - 2, P], BF16)  # one-hot unpackers
        nc.sync.dma_start(selA[:], t["selA"][:])
        selB = persist.tile([2, 2, P], BF16)
        nc.sync.dma_start(selB[:], t["selB"][:])
        packA = persist.tile([1, NH - 2, NH - 2], BF16)
        nc.sync.dma_start(packA[:], t["packA"][:])
        packB = persist.tile([1, 2, 2], BF16)
        nc.sync.dma_start(packB[:], t["packB"][:])

        def colnorm_rows(pool, psum_pool, ss_ps, inv_dim):
            """[1,n] sum-of-squares PSUM -> [128,n] SBUF rsqrt(mean+eps)."""
            n = ss_ps.shape[-1]
            srow = pool.tile([1, n], F32, tag="srow")
            nc.scalar.activation(srow[:], ss_ps[:], SQRT,
                                 bias=eps1[:], scale=inv_dim)
            rrow = pool.tile([1, n], F32R, tag="rrow")
            with nc.allow_low_precision(reason="f32r is f32 storage"):
                nc.vector.reciprocal(rrow[:], srow[:])
            bc_ps = psum_pool.tile([P, n], F32, tag="bcast")
            _mm(nc, bc_ps[:], ones_row[:], rrow[:], True, True)
            bc = pool.tile([P, n], F32, tag="bcn")
            nc.scalar.activation(bc[:], bc_ps[:], COPY)
            return bc

        # weight tiles for attention group 0 + first o_proj blocks are
        # prefetched long before their phases to keep the PE fed at the
        # C->D and D->E boundaries.
        pdw = big.enter_context(tc.tile_pool(name="phD_w", bufs=2))
        pew = big.enter_context(tc.tile_pool(name="phE_w", bufs=3))
        wv0 = pdw.tile([P, KB_CKV, 2 * DV], BF16, tag="wv")
        nc.sync.dma_start(wv0[:], t["wv_t"][0])
        wkn0 = pdw.tile([P, KB_CKV, DN], BF16, tag="wkn")
        nc.sync.dma_start(wkn0[:], t["wkn_t"][0])
        wkn1 = pdw.tile([P, KB_CKV, DN], BF16, tag="wkn")
        nc.sync.dma_start(wkn1[:], t["wkn_t"][1])

        # ---------------- phases A-C in their own SBUF scope --------------
        with tc.tile_pool(name="lns", bufs=1) as plns, \
             tc.tile_pool(name="hn", bufs=2) as pbh, \
             tc.tile_pool(name="ropec", bufs=1) as ropec, \
             tc.tile_pool(name="qaTp", bufs=1) as paq, \
             tc.tile_pool(name="phA", bufs=2) as pa, \
             tc.tile_pool(name="rope1", bufs=2) as prot, \
             tc.tile_pool(name="psA", bufs=3, space="PSUM") as psA, \
             tc.tile_pool(name="psSS", bufs=4, space="PSUM") as psSS, \
             tc.tile_pool(name="psBC", bufs=1, space="PSUM") as psBC:

            qa_ln = plns.tile([P, KB_QLR], F32)
            nc.sync.dma_start(qa_ln[:], t["qa_ln_p"][:])
            kva_ln = plns.tile([P, KB_CKV], F32)
            nc.sync.dma_start(kva_ln[:], t["kva_ln_p"][:])

            # prefetches issued before phase A compute
            hn_cur = pbh.tile([P, KB_HID, W], BF16, tag="hn")
            nc.sync.dma_start(hn_cur[:, :KB_HID // 2, :],
                              t["hsT_t"][:, :KB_HID // 2, 0:W])
            nc.sync.dma_start(hn_cur[:, KB_HID // 2:, :],
                              t["hsT_t"][:, KB_HID // 2:, 0:W])
            cosf = ropec.tile([DR, S], BF16)
            nc.sync.dma_start(cosf[:], t["cosf64"][:])
            sinsf = ropec.tile([DR, S], BF16)
            nc.sync.dma_start(sinsf[:], t["sinsf64"][:])
            cos2p = ropec.tile([P, W], BF16)
            nc.sync.dma_start(cos2p[:], t["cos2p"][:])
            sin2sp = ropec.tile([P, W], BF16)
            nc.sync.dma_start(sin2sp[:], t["sin2sp"][:])

            qaT = paq.tile([P, KB_QLR, W], BF16)

            # ---- phase A: qaT panel + rmsnorm (SBUF-resident) ----
            with tc.tile_pool(name="phA_h", bufs=1) as pah, \
                 tc.tile_pool(name="wA", bufs=2) as paw:
                hp = pah.tile([P, KB_HID, W], BF16, tag="hp")
                for q4 in range(4):
                    nc.sync.dma_start(hp[:, 4 * q4:4 * (q4 + 1), :],
                                      t["hs_panel_t"][:, 4 * q4:4 * (q4 + 1), :])
                ssq = psSS.tile([1, W], F32, tag="ss")
                for m in range(MB_QLR):
                    wm = paw.tile([P, KB_HID, P], BF16, tag="wqa")
                    nc.sync.dma_start(wm[:], t["w_qa_t"][m])
                    ps = psA.tile([P, W], F32, tag="psA")
                    for k in range(KB_HID):
                        _mm(nc, ps[:], wm[:, k, :], hp[:, k, :],
                            k == 0, k == KB_HID - 1)
                    nc.scalar.activation(qaT[:, m, :], ps[:], COPY)
                    sq = pa.tile([P, W], F32R, tag="sq")
                    nc.vector.tensor_tensor(sq[:], qaT[:, m, :], ps[:], MULT)
                    _mm(nc, ssq[:], ones_col[:], sq[:],
                        m == 0, m == MB_QLR - 1)

                def a_norm():
                    rq = colnorm_rows(pa, psBC, ssq[:], 1.0 / QLR)
                    for m in range(MB_QLR):
                        nc.vector.scalar_tensor_tensor(
                            qaT[:, m, :], qaT[:, m, :], qa_ln[:, m:m + 1],
                            rq[:], MULT, MULT)
                # deferred rmsnorm tails: each closure emits the bcast +
                # scale for an earlier block, placed after later matmuls so
                # the PE never waits on the sqrt/reciprocal chain.
                pending = [a_norm]

            # ---- phase B: ckT (full S) + rmsnorm + kpe rope ----
            with tc.tile_pool(name="wB", bufs=MB_KVA) as pbw, \
                 tc.tile_pool(name="kraw", bufs=1) as pkr:
                kraw = pkr.tile([DR, S], BF16)
                wkv = []
                for m in range(MB_KVA):
                    wt = pbw.tile([P, KB_HID, P], BF16, tag="wkva")
                    nc.sync.dma_start(wt[:], t["w_kva_t"][m])
                    wkv.append(wt)
                for nch in range(NCH):
                    if nch + 1 < NCH:
                        hn_next = pbh.tile([P, KB_HID, W], BF16, tag="hn")
                        nc.sync.dma_start(
                            hn_next[:, :KB_HID // 2, :],
                            t["hsT_t"][:, :KB_HID // 2,
                                       (nch + 1) * W:(nch + 2) * W])
                        nc.sync.dma_start(
                            hn_next[:, KB_HID // 2:, :],
                            t["hsT_t"][:, KB_HID // 2:,
                                       (nch + 1) * W:(nch + 2) * W])
                    ss = psSS.tile([1, W], F32, tag="ss")
                    for m in range(MB_KVA):
                        ps = psA.tile([P, W], F32, tag="psA")
                        for k in range(KB_HID):
                            _mm(nc, ps[:], wkv[m][:, k, :], hn_cur[:, k, :],
                                k == 0, k == KB_HID - 1)
                        if m < KB_CKV:
                            ckslc = ckT[:, m, nch * W:(nch + 1) * W]
                            nc.scalar.activation(ckslc, ps[:], COPY)
                            sq = pa.tile([P, W], F32R, tag="sq")
                            nc.vector.tensor_tensor(sq[:], ckslc, ps[:],
                                                    MULT)
                            _mm(nc, ss[:], ones_col[:], sq[:],
                                m == 0, m == KB_CKV - 1)
                        else:
                            nc.vector.tensor_copy(
                                kraw[:, nch * W:(nch + 1) * W], ps[0:DR, :])
                    if nch >= 2 and pending:
                        pending.pop(0)()

                    def b_norm(ss=ss, nch=nch):
                        rk = colnorm_rows(pa, psBC, ss[:], 1.0 / KVLR)
                        for m in range(KB_CKV):
                            nc.vector.scalar_tensor_tensor(
                                ckT[:, m, nch * W:(nch + 1) * W],
                                ckT[:, m, nch * W:(nch + 1) * W],
                                kva_ln[:, m:m + 1], rk[:], MULT, MULT)
                    pending.append(b_norm)
                    if nch + 1 < NCH:
                        hn_cur = hn_next
                # RoPE on kraw [64, S] -> kpe64 (bf16)
                rot = pkr.tile([DR, S], BF16, tag="rotk")
                nc.vector.tensor_copy(rot[0:DR // 2, :], kraw[DR // 2:DR, :])
                nc.vector.tensor_copy(rot[DR // 2:DR, :], kraw[0:DR // 2, :])
                nc.vector.tensor_tensor(kraw[:], kraw[:], cosf[:], MULT)
                nc.vector.tensor_tensor(rot[:], rot[:], sinsf[:], MULT)
                nc.vector.tensor_tensor(kpe64[:], kraw[:], rot[:], ADD)

            # ---- phase C: qT panel (+ RoPE on pe part) ----
            with tc.tile_pool(name="wC", bufs=2) as pcw:
                for m in range(MB_NOPE + MB_PE):
                    wm = pcw.tile([P, KB_QLR, P], BF16, tag="wqb")
                    nc.sync.dma_start(wm[:, :KB_QLR // 2, :],
                                      t["w_qb_t"][m, :, :KB_QLR // 2, :])
                    nc.sync.dma_start(wm[:, KB_QLR // 2:, :],
                                      t["w_qb_t"][m, :, KB_QLR // 2:, :])
                    if pending:
                        pending.pop(0)()
                    ps = psA.tile([P, W], F32, tag="psA")
                    for k in range(KB_QLR):
                        _mm(nc, ps[:], wm[:, k, :], qaT[:, k, :],
                            k == 0, k == KB_QLR - 1)
                    if m < MB_NOPE:
                        nc.scalar.activation(qnopeT[:, m, :], ps[:], COPY)
                    else:
                        j = m - MB_NOPE
                        rotq = prot.tile([P, W], F32, tag="rotq")
                        for h in (0, DR):
                            nc.vector.tensor_copy(rotq[h:h + 32, :],
                                                  ps[h + 32:h + 64, :])
                            nc.vector.tensor_copy(rotq[h + 32:h + 64, :],
                                                  ps[h:h + 32, :])
                        nc.vector.tensor_tensor(rotq[:], rotq[:], sin2sp[:],
                                                MULT)
                        qpe2 = prot.tile([P, W], F32, tag="qpe2")
                        nc.vector.tensor_tensor(qpe2[:], ps[:], cos2p[:],
                                                MULT)
                        nc.vector.tensor_tensor(qpe2[:], qpe2[:], rotq[:],
                                                ADD)
                        nc.vector.tensor_copy(qpe64[:, 2 * j, :],
                                              qpe2[0:DR, :])
                        nc.vector.tensor_copy(qpe64[:, 2 * j + 1, :],
                                              qpe2[DR:P, :])

        # ---------------- phase D: attention per 2-head group -------------
        with tc.tile_pool(name="phD_v", bufs=2) as pdv, \
             tc.tile_pool(name="phD_k", bufs=2) as pdk, \
             tc.tile_pool(name="pracc", bufs=3) as pacc, \
             tc.tile_pool(name="prrow", bufs=3) as prow, \
             tc.tile_pool(name="probs", bufs=4) as pprob, \
             tc.tile_pool(name="psSc", bufs=3, space="PSUM") as psSc, \
             tc.tile_pool(name="psO", bufs=2, space="PSUM") as psO, \
             tc.tile_pool(name="psR", bufs=1, space="PSUM") as psR, \
             tc.tile_pool(name="psPk", bufs=2, space="PSUM") as psPk, \
             ExitStack() as dctx:
            if with_mask:
                mask_pool = dctx.enter_context(
                    tc.tile_pool(name="maskp", bufs=4))

            packA_ps = psPk.tile([NH - 2, W], F32, tag="pk")
            packB_ps = psPk.tile([P, W], F32, tag="pk")

            def head_tail(h, pracc_t, po):
                """denominator reduce, pack into row table, drain po."""
                pr = psR.tile([1, W], F32, tag="pr")
                _mm(nc, pr[:], ones_col[:], pracc_t[:], True, True)
                prow_t = prow.tile([1, W], BF16, tag="prw")
                nc.scalar.activation(prow_t[:], pr[:], COPY)
                if h < NH - 2:
                    _mm(nc, packA_ps[:], packA[:, h, :], prow_t[:],
                        h == 0, h == NH - 3)
                else:
                    _mm(nc, packB_ps[:2, :], packB[:, h - (NH - 2), :],
                        prow_t[:], h == NH - 2, h == NH - 1)
                nc.scalar.activation(oT[:, h, :], po[:], COPY)

            def apply_norm(h):
                """oT[:,h,:] *= broadcast(1/pr_h) via one-hot unpack."""
                bc = psPk.tile([P, W], F32, tag="pk")
                if h < NH - 2:
                    _mm(nc, bc[:], selA[:, h, :], prrA[:], True, True)
                else:
                    _mm(nc, bc[:], selB[:, h - (NH - 2), :], prrB[:],
                        True, True)
                nc.vector.tensor_tensor(oT[:, h, :], oT[:, h, :], bc[:],
                                        MULT)

            for g in range(NH // 2):
                h0, h1 = 2 * g, 2 * g + 1
                # V for the 2 heads: v_sb[k, 2*128]
                if g == 0:
                    wv = wv0
                else:
                    wv = pdw.tile([P, KB_CKV, 2 * DV], BF16, tag="wv")
                    nc.sync.dma_start(wv[:], t["wv_t"][g])
                v_sb = pdv.tile([P, KB_S, 2 * DV], BF16, tag="v")
                for kb in range(KB_S):
                    psv = psSc.tile([P, W], F32, tag="pss")
                    for kc in range(KB_CKV):
                        _mm(nc, psv[:, :2 * DV],
                            ckT[:, kc, kb * P:(kb + 1) * P],
                            wv[:, kc, :], kc == 0, kc == KB_CKV - 1)
                    nc.vector.tensor_copy(v_sb[:, kb, :], psv[:, :2 * DV])

                # knopeT for both heads: [128 d, S] each
                knT = []
                for hl in range(2):
                    h = h0 + hl
                    if g == 0:
                        wkn = wkn0 if hl == 0 else wkn1
                    else:
                        wkn = pdw.tile([P, KB_CKV, DN], BF16, tag="wkn")
                        nc.sync.dma_start(wkn[:], t["wkn_t"][h])
                    kn = pdk.tile([P, KB_S, P], BF16, tag="knT")
                    for nch in range(NCH):
                        psk = psSc.tile([P, W], F32, tag="pss")
                        for kc in range(KB_CKV):
                            _mm(nc, psk[:], wkn[:, kc, :],
                                ckT[:, kc, nch * W:(nch + 1) * W],
                                kc == 0, kc == KB_CKV - 1)
                        nc.vector.tensor_copy(
                            kn[:, nch * (W // P):(nch + 1) * (W // P), :],
                            psk[:])
                    knT.append(kn)

                if g == NH // 2 - 1:
                    # prefetch the first o_proj weight tiles under the
                    # last attention group
                    wo_pre = []
                    for m in range(3):
                        wt = pew.tile([P, NH, P], BF16, tag="wo")
                        nc.sync.dma_start(wt[:], t["w_o_t"][m])
                        wo_pre.append(wt)

                # attention: both heads interleaved over key blocks
                po0 = psO.tile([P, W], F32, tag="po")
                po1 = psO.tile([P, W], F32, tag="po")
                pra0 = pacc.tile([P, W], F32R, tag="pra")
                pra1 = pacc.tile([P, W], F32R, tag="pra")
                for kb in range(KB_S):
                    kbs = slice(kb * P, (kb + 1) * P)
                    pss0 = psSc.tile([P, W], F32, tag="pss")
                    pss1 = psSc.tile([P, W], F32, tag="pss")
                    _mm(nc, pss0[:], knT[0][:, kb, :], qnopeT[:, h0, :],
                        True, False)
                    _mm(nc, pss1[:], knT[1][:, kb, :], qnopeT[:, h1, :],
                        True, False)
                    _mm(nc, pss0[:], kpe64[:, kbs], qpe64[:, h0, :],
                        False, True)
                    _mm(nc, pss1[:], kpe64[:, kbs], qpe64[:, h1, :],
                        False, True)
                    for hl, pss, po, pra in ((0, pss0, po0, pra0),
                                             (1, pss1, po1, pra1)):
                        probs = pprob.tile([P, W], BF16, tag="probs")
                        if with_mask:
                            mtile = mask_pool.tile([P, W], F32, tag="mt")
                            nc.sync.dma_start(mtile[:], t["maskT"][kbs, :])
                            psc = pacc.tile([P, W], F32R, tag="msc")
                            nc.vector.scalar_tensor_tensor(
                                psc[:], pss[:], SCALE, mtile[:], MULT, ADD)
                            nc.scalar.activation(probs[:], psc[:], EXP)
                        else:
                            nc.scalar.activation(probs[:], pss[:], EXP,
                                                 scale=SCALE)
                        _mm(nc, po[:], v_sb[:, kb, hl * DV:(hl + 1) * DV],
                            probs[:], kb == 0, kb == KB_S - 1)
                        if kb == 0:
                            nc.vector.tensor_copy(pra[:], probs[:])
                        else:
                            nc.vector.tensor_tensor(pra[:], pra[:],
                                                    probs[:], ADD)
                head_tail(h0, pra0, po0)
                head_tail(h1, pra1, po1)

                if h1 == NH - 3:
                    # heads 0..13 packed: their batched reciprocal runs
                    # under the last group's attention.
                    nc.scalar.activation(prallA[:], packA_ps[:], COPY)
                    with nc.allow_low_precision(reason="f32r is f32 storage"):
                        nc.vector.reciprocal(prrA[:], prallA[:])

            nc.scalar.activation(prallB[:], packB_ps[:2, :], COPY)
            with nc.allow_low_precision(reason="f32r is f32 storage"):
                nc.vector.reciprocal(prrB[:], prallB[:])
            for h in range(NH):
                apply_norm(h)

        # ---------------- phase E: o_proj ---------------------------------
        with tc.tile_pool(name="phE", bufs=2) as pe, \
             tc.tile_pool(name="psE", bufs=2, space="PSUM") as psE:
            for m in range(MB_HID):
                if m < 3:
                    wm = wo_pre[m]
                else:
                    wm = pew.tile([P, NH, P], BF16, tag="wo")
                    nc.sync.dma_start(wm[:], t["w_o_t"][m])
                ps = psE.tile([P, W], F32, tag="psE")
                for k in range(NH):
                    _mm(nc, ps[:], wm[:, k, :], oT[:, k, :],
                        k == 0, k == NH - 1)
                osb = pe.tile([P, W], F32, tag="osb")
                nc.scalar.activation(osb[:], ps[:], COPY)
                nc.sync.dma_start(t["outT"][:, m, :], osb[:])


def _build_program(with_mask):
    nc = bacc.Bacc("TRN2", target_bir_lowering=False, debug=False)
    t = {}

    def inp(name, shape, dt=F32):
        t[name] = nc.dram_tensor(name, list(shape), dt,
                                 kind="ExternalInput").ap()

    inp("hsT_t", [P, KB_HID, S], BF16)
    inp("hs_panel_t", [P, KB_HID, W], BF16)
    inp("w_qa_t", [MB_QLR, P, KB_HID, P], BF16)
    inp("w_qb_t", [MB_NOPE + MB_PE, P, KB_QLR, P], BF16)
    inp("w_kva_t", [MB_KVA, P, KB_HID, P], BF16)
    inp("wv_t", [NH // 2, P, KB_CKV, 2 * DV], BF16)
    inp("wkn_t", [NH, P, KB_CKV, DN], BF16)
    inp("w_o_t", [MB_HID, P, NH, P], BF16)
    inp("qa_ln_p", [P, KB_QLR])
    inp("kva_ln_p", [P, KB_CKV])
    inp("cos2p", [P, W], BF16)
    inp("sin2sp", [P, W], BF16)
    inp("cosf64", [DR, S], BF16)
    inp("sinsf64", [DR, S], BF16)
    inp("selA", [NH - 2, NH - 2, P], BF16)
    inp("selB", [2, 2, P], BF16)
    inp("packA", [1, NH - 2, NH - 2], BF16)
    inp("packB", [1, 2, 2], BF16)
    if with_mask:
        inp("maskT", [S, W])
    t["outT"] = nc.dram_tensor("outT", [P, MB_HID, W], F32,
                               kind="ExternalOutput").ap()

    with tile.TileContext(nc) as tc:
        _emit(tc, t, with_mask)
    nc.compile()
    return nc


_PROG_CACHE = {}


def _get_program(with_mask):
    if with_mask not in _PROG_CACHE:
        _PROG_CACHE[with_mask] = _build_program(with_mask)
    return _PROG_CACHE[with_mask]


def _tile_w(w, colw=P):
    """[R, C] -> [C//colw, 128, R//128, colw] bf16 contiguous."""
    R, C = w.shape
    kb = R // P
    mb = C // colw
    # wt[m, p, k, c] = w[k*128+p, m*colw+c]
    wt = w.reshape(kb, P, mb, colw).transpose(2, 1, 0, 3)
    return np.ascontiguousarray(wt.astype(ml_dtypes.bfloat16))


def make_in_maps(hidden_states, attention_mask, cos, sin, w_qa, qa_ln, w_qb,
                 w_kva, kva_ln, w_kvb, w_o, with_mask):
    f32 = np.float32
    c = np.ascontiguousarray

    w_qb_r = np.asarray(w_qb).reshape(QLR, NH, DQK)
    w_qb_re = np.concatenate(
        [w_qb_r[:, :, :DN].reshape(QLR, NH * DN),
         w_qb_r[:, :, DN:].reshape(QLR, NH * DR)], axis=1).astype(f32)
    w_kvb_r = np.asarray(w_kvb).reshape(KVLR, NH, DN + DV)
    w_kn = w_kvb_r[:, :, :DN].reshape(KVLR, NH * DN).astype(f32)
    w_v = w_kvb_r[:, :, DN:].reshape(KVLR, NH * DV).astype(f32)
    w_kva_pad = np.zeros((HID, MB_KVA * P), f32)
    w_kva_pad[:, :KVLR + DR] = np.asarray(w_kva).astype(f32)

    qa_ln_p = c(np.asarray(qa_ln).reshape(KB_QLR, P).T.astype(f32))
    kva_ln_p = c(np.asarray(kva_ln).reshape(KB_CKV, P).T.astype(f32))

    cosT = np.asarray(cos).T.astype(f32)                  # [64, S]
    sinT = np.asarray(sin).T.astype(f32)
    sin_s = np.concatenate([-sinT[:DR // 2], sinT[DR // 2:]], axis=0)
    cos2 = np.concatenate([cosT, cosT], axis=0)           # [128, S]
    sin2s = np.concatenate([sin_s, sin_s], axis=0)

    shared = {
        "w_qa_t": _tile_w(np.asarray(w_qa).astype(f32)),
        "w_qb_t": _tile_w(w_qb_re),
        "w_kva_t": _tile_w(w_kva_pad),
        "wv_t": _tile_w(w_v, colw=2 * DV),
        "wkn_t": _tile_w(w_kn, colw=DN),
        "w_o_t": _tile_w(np.asarray(w_o).astype(f32)),
        "qa_ln_p": qa_ln_p,
        "kva_ln_p": kva_ln_p,
        "cosf64": c(cosT.astype(ml_dtypes.bfloat16)),
        "sinsf64": c(sin_s.astype(ml_dtypes.bfloat16)),
        "selA": c((np.eye(NH - 2, dtype=f32)[:, :, None]
                   * np.ones((1, 1, P), f32)).astype(ml_dtypes.bfloat16)),
        "selB": c((np.eye(2, dtype=f32)[:, :, None]
                   * np.ones((1, 1, P), f32)).astype(ml_dtypes.bfloat16)),
        "packA": c(np.eye(NH - 2, dtype=f32)[None].astype(ml_dtypes.bfloat16)),
        "packB": c(np.eye(2, dtype=f32)[None].astype(ml_dtypes.bfloat16)),
    }

    hs = np.asarray(hidden_states)
    am = np.asarray(attention_mask)
    in_maps = []
    for core in range(NCORES):
        b, pnl = divmod(core, NPANEL)
        q0 = pnl * W
        # hsT_t[p, k, s] = hs[b, s, k*128+p]
        hsT_t = np.asarray(hs[b]).astype(ml_dtypes.bfloat16) \
            .reshape(S, KB_HID, P).transpose(2, 1, 0)
        m = dict(shared)
        m["hsT_t"] = c(hsT_t)
        m["hs_panel_t"] = c(hsT_t[:, :, q0:q0 + W])
        m["cos2p"] = c(cos2[:, q0:q0 + W].astype(ml_dtypes.bfloat16))
        m["sin2sp"] = c(sin2s[:, q0:q0 + W].astype(ml_dtypes.bfloat16))
        if with_mask:
            m["maskT"] = c(am[b, 0, q0:q0 + W, :].T.astype(f32))
        in_maps.append(m)
    return in_maps


def kernel(hidden_states, attention_mask, cos, sin, w_qa, qa_ln, w_qb,
           w_kva, kva_ln, w_kvb, w_o):
    global LAST_RESULT
    with_mask = bool(np.any(np.asarray(attention_mask) != 0))
    nc = _get_program(with_mask)
    in_maps = make_in_maps(hidden_states, attention_mask, cos, sin, w_qa,
                           qa_ln, w_qb, w_kva, kva_ln, w_kvb, w_o, with_mask)
    trace = os.environ.get("KERNEL_TRACE", "0") == "1"
    res = bass_utils.run_bass_kernel_spmd(
        nc, in_maps, core_ids=list(range(NCORES)), trace=trace)
    LAST_RESULT = res

    out = np.empty((B, S, HID), np.float32)
    for core in range(NCORES):
        b, pnl = divmod(core, NPANEL)
        q0 = pnl * W
        # outT[p, m, s] -> out[b, q0+s, m*128+p]
        out[b, q0:q0 + W, :] = res.results[core]["outT"] \
            .transpose(2, 1, 0).reshape(W, HID)
    return out


# revision 34
# speedup vs baseline: 1.0185x; 1.0185x over previous
"""DeepseekV2 MLA attention forward — Trainium2 Bass kernel (8 NeuronCores).

Sharding: data-parallel over batch (2) x sequence-parallel over query rows
(4 panels of 512) = 8 cores, no cross-core communication.  Each core:
  - q path (q_a_proj -> rmsnorm -> q_b_proj) for its 512 query rows
  - kv path (kv_a_proj -> rmsnorm -> kv_b_proj) for the FULL key sequence
  - RoPE, full attention (16 heads), o_proj for its query rows

Schedule notes (v2):
  - every matmul is bf16 x bf16 with fp32 PSUM accumulation (weights are
    pre-tiled contiguously on the host; activations are written back from
    PSUM as bf16).  FWL halves LDWEIGHTS; DMA and SBUF traffic halve.
  - qa / oT live in SBUF across phases (no DRAM round trip).
  - softmax denominators accumulate on DVE (no PE reduce per key block) into
    a [16, W] table; one batched reciprocal for heads 0..14 runs under the
    last group's attention, so only head 15's reciprocal is exposed before
    o_proj starts.  The numerator and denominator consume the same rounded
    bf16 probs, so the bf16 error largely cancels in the ratio.
  - q_pe lives on partitions 0:63 for all 16 heads so every rope-score
    matmul of a key block shares the same k_pe stationary tile.
"""

import os
import numpy as np
from contextlib import ExitStack

import ml_dtypes

import concourse.bass as bass
import concourse.bacc as bacc
import concourse.mybir as mybir
import concourse.tile as tile
from concourse import bass_utils

B, S, HID = 2, 2048, 2048
NH = 16
QLR, KVLR = 1536, 512
DN, DR, DV = 128, 64, 128
DQK = DN + DR
SCALE = DQK ** -0.5
EPS = 1e-6
P = 128
NPANEL = 4
W = S // NPANEL            # 512 query rows per core
NCORES = B * NPANEL

F32 = mybir.dt.float32
F32R = mybir.dt.float32r
BF16 = mybir.dt.bfloat16
EXP = mybir.ActivationFunctionType.Exp
SQRT = mybir.ActivationFunctionType.Sqrt
COPY = mybir.ActivationFunctionType.Copy
MULT = mybir.AluOpType.mult
ADD = mybir.AluOpType.add

KB_HID = HID // P          # 16
KB_QLR = QLR // P          # 12
KB_CKV = KVLR // P         # 4
KB_S = S // P              # 16
MB_QLR = QLR // P          # 12
MB_NOPE = NH * DN // P     # 16
MB_PE = NH * DR // P       # 8
MB_HID = HID // P          # 16
MB_KVA = 5                 # w_kva padded 576 -> 640 columns
NCH = S // W               # 4 column chunks of the full sequence

LAST_RESULT = None


def _mm(nc, out, lhsT, rhs, start, stop):
    l = lhsT if lhsT.dtype == BF16 else lhsT.bitcast(F32R)
    r = rhs if rhs.dtype == BF16 else rhs.bitcast(F32R)
    nc.tensor.matmul(out, l, r, start=start, stop=stop)


def _emit(tc, t, with_mask):
    nc = tc.nc

    with ExitStack() as big:
        const = big.enter_context(tc.tile_pool(name="const", bufs=1))
        ones_cf = const.tile([P, 1], F32)
        nc.vector.memset(ones_cf[:], 1.0)
        ones_rf = const.tile([1, P], F32)
        nc.vector.memset(ones_rf[:], 1.0)
        ones_col = const.tile([P, 1], F32R)
        nc.scalar.activation(ones_col[:], ones_cf[:], COPY)
        ones_row = const.tile([1, P], F32R)
        nc.scalar.activation(ones_row[:], ones_rf[:], COPY)
        eps1 = const.tile([1, 1], F32)
        nc.vector.memset(eps1[:], EPS)

        # persistent activation tiles (live into phase D/E)
        persist = big.enter_context(tc.tile_pool(name="persist", bufs=1))
        ckT = persist.tile([P, KB_CKV, S], BF16)       # 16 KB/part
        kpe64 = persist.tile([DR, S], BF16)            # k_pe, roped
        qnopeT = persist.tile([P, MB_NOPE, W], BF16)   # 16 KB/part
        qpe64 = persist.tile([DR, NH, W], BF16)        # q_pe, all heads, 0:63
        oT = persist.tile([P, NH, W], BF16)            # 16 KB/part
        prallA = persist.tile([NH - 2, W], F32)        # denominators h 0..13
        prrA = persist.tile([NH - 2, W], BF16)
        prallB = persist.tile([2, W], F32)             # denominators h 14,15
        prrB = persist.tile([2, W], BF16)
        selA = persist.tile([NH - 2, NH - 2, P], BF16)  # one-hot unpackers
        nc.sync.dma_start(selA[:], t["selA"][:])
        selB = persist.tile([2, 2, P], BF16)
        nc.sync.dma_start(selB[:], t["selB"][:])
        packA = persist.tile([1, NH - 2, NH - 2], BF16)
        nc.sync.dma_start(packA[:], t["packA"][:])
        packB = persist.tile([1, 2, 2], BF16)
        nc.sync.dma_start(packB[:], t["packB"][:])

        def colnorm_rows(pool, psum_pool, ss_ps, inv_dim):
            """[1,n] sum-of-squares PSUM -> [128,n] SBUF rsqrt(mean+eps)."""
            n = ss_ps.shape[-1]
            srow = pool.tile([1, n], F32, tag="srow")
            nc.scalar.activation(srow[:], ss_ps[:], SQRT,
                                 bias=eps1[:], scale=inv_dim)
            rrow = pool.tile([1, n], F32R, tag="rrow")
            with nc.allow_low_precision(reason="f32r is f32 storage"):
                nc.vector.reciprocal(rrow[:], srow[:])
            bc_ps = psum_pool.tile([P, n], F32, tag="bcast")
            _mm(nc, bc_ps[:], ones_row[:], rrow[:], True, True)
            bc = pool.tile([P, n], F32, tag="bcn")
            nc.scalar.activation(bc[:], bc_ps[:], COPY)
            return bc

        # weight tiles for attention group 0 + first o_proj blocks are
        # prefetched long before their phases to keep the PE fed at the
        # C->D and D->E boundaries.
        pdw = big.enter_context(tc.tile_pool(name="phD_w", bufs=2))
        pew = big.enter_context(tc.tile_pool(name="phE_w", bufs=3))
        wv0 = pdw.tile([P, KB_CKV, 2 * DV], BF16, tag="wv")
        nc.sync.dma_start(wv0[:], t["wv_t"][0])
        wkn0 = pdw.tile([P, KB_CKV, DN], BF16, tag="wkn")
        nc.sync.dma_start(wkn0[:], t["wkn_t"][0])
        wkn1 = pdw.tile([P, KB_CKV, DN], BF16, tag="wkn")
        nc.sync.dma_start(wkn1[:], t["wkn_t"][1])

        # ---------------- phases A-C in their own SBUF scope --------------
        with tc.tile_pool(name="lns", bufs=1) as plns, \
             tc.tile_pool(name="hn", bufs=2) as pbh, \
             tc.tile_pool(name="ropec", bufs=1) as ropec, \
             tc.tile_pool(name="qaTp", bufs=1) as paq, \
             tc.tile_pool(name="phA", bufs=2) as pa, \
             tc.tile_pool(name="rope1", bufs=2) as prot, \
             tc.tile_pool(name="psA", bufs=3, space="PSUM") as psA, \
             tc.tile_pool(name="psSS", bufs=3, space="PSUM") as psSS, \
             tc.tile_pool(name="psBC", bufs=1, space="PSUM") as psBC:

            qa_ln = plns.tile([P, KB_QLR], F32)
            nc.sync.dma_start(qa_ln[:], t["qa_ln_p"][:])
            kva_ln = plns.tile([P, KB_CKV], F32)
            nc.sync.dma_start(kva_ln[:], t["kva_ln_p"][:])

            # prefetches issued before phase A compute
            hn_cur = pbh.tile([P, KB_HID, W], BF16, tag="hn")
            nc.sync.dma_start(hn_cur[:, :KB_HID // 2, :],
                              t["hsT_t"][:, :KB_HID // 2, 0:W])
            nc.sync.dma_start(hn_cur[:, KB_HID // 2:, :],
                              t["hsT_t"][:, KB_HID // 2:, 0:W])
            cosf = ropec.tile([DR, S], BF16)
            nc.sync.dma_start(cosf[:], t["cosf64"][:])
            sinsf = ropec.tile([DR, S], BF16)
            nc.sync.dma_start(sinsf[:], t["sinsf64"][:])
            cos2p = ropec.tile([P, W], BF16)
            nc.sync.dma_start(cos2p[:], t["cos2p"][:])
            sin2sp = ropec.tile([P, W], BF16)
            nc.sync.dma_start(sin2sp[:], t["sin2sp"][:])

            qaT = paq.tile([P, KB_QLR, W], BF16)

            # ---- phase A: qaT panel + rmsnorm (SBUF-resident) ----
            with tc.tile_pool(name="phA_h", bufs=1) as pah, \
                 tc.tile_pool(name="wA", bufs=2) as paw:
                hp = pah.tile([P, KB_HID, W], BF16, tag="hp")
                for q4 in range(4):
                    nc.sync.dma_start(hp[:, 4 * q4:4 * (q4 + 1), :],
                                      t["hs_panel_t"][:, 4 * q4:4 * (q4 + 1), :])
                ssq = psSS.tile([1, W], F32, tag="ss")
                for m in range(MB_QLR):
                    wm = paw.tile([P, KB_HID, P], BF16, tag="wqa")
                    nc.sync.dma_start(wm[:], t["w_qa_t"][m])
                    ps = psA.tile([P, W], F32, tag="psA")
                    for k in range(KB_HID):
                        _mm(nc, ps[:], wm[:, k, :], hp[:, k, :],
                            k == 0, k == KB_HID - 1)
                    nc.scalar.activation(qaT[:, m, :], ps[:], COPY)
                    sq = pa.tile([P, W], F32R, tag="sq")
                    nc.vector.tensor_tensor(sq[:], qaT[:, m, :], ps[:], MULT)
                    _mm(nc, ssq[:], ones_col[:], sq[:],
                        m == 0, m == MB_QLR - 1)

                def a_norm():
                    rq = colnorm_rows(pa, psBC, ssq[:], 1.0 / QLR)
                    for m in range(MB_QLR):
                        nc.vector.scalar_tensor_tensor(
                            qaT[:, m, :], qaT[:, m, :], qa_ln[:, m:m + 1],
                            rq[:], MULT, MULT)
                # deferred rmsnorm tails: each closure emits the bcast +
                # scale for an earlier block, placed after later matmuls so
                # the PE never waits on the sqrt/reciprocal chain.
                pending = [a_norm]

            # ---- phase B: ckT (full S) + rmsnorm + kpe rope ----
            with tc.tile_pool(name="wB", bufs=MB_KVA) as pbw, \
                 tc.tile_pool(name="kraw", bufs=1) as pkr:
                kraw = pkr.tile([DR, S], BF16)
                wkv = []
                for m in range(MB_KVA):
                    wt = pbw.tile([P, KB_HID, P], BF16, tag="wkva")
                    nc.sync.dma_start(wt[:], t["w_kva_t"][m])
                    wkv.append(wt)
                for nch in range(NCH):
                    if nch + 1 < NCH:
                        hn_next = pbh.tile([P, KB_HID, W], BF16, tag="hn")
                        nc.sync.dma_start(
                            hn_next[:, :KB_HID // 2, :],
                            t["hsT_t"][:, :KB_HID // 2,
                                       (nch + 1) * W:(nch + 2) * W])
                        nc.sync.dma_start(
                            hn_next[:, KB_HID // 2:, :],
                            t["hsT_t"][:, KB_HID // 2:,
                                       (nch + 1) * W:(nch + 2) * W])
                    ss = psSS.tile([1, W], F32, tag="ss")
                    for m in range(MB_KVA):
                        ps = psA.tile([P, W], F32, tag="psA")
                        for k in range(KB_HID):
                            _mm(nc, ps[:], wkv[m][:, k, :], hn_cur[:, k, :],
                                k == 0, k == KB_HID - 1)
                        if m < KB_CKV:
                            ckslc = ckT[:, m, nch * W:(nch + 1) * W]
                            nc.scalar.activation(ckslc, ps[:], COPY)
                            sq = pa.tile([P, W], F32R, tag="sq")
                            nc.vector.tensor_tensor(sq[:], ckslc, ps[:],
                                                    MULT)
                            _mm(nc, ss[:], ones_col[:], sq[:],
                                m == 0, m == KB_CKV - 1)
                        else:
                            nc.vector.tensor_copy(
                                kraw[:, nch * W:(nch + 1) * W], ps[0:DR, :])
                    if nch >= 1 and pending:
                        pending.pop(0)()

                    def b_norm(ss=ss, nch=nch):
                        rk = colnorm_rows(pa, psBC, ss[:], 1.0 / KVLR)
                        for m in range(KB_CKV):
                            nc.vector.scalar_tensor_tensor(
                                ckT[:, m, nch * W:(nch + 1) * W],
                                ckT[:, m, nch * W:(nch + 1) * W],
                                kva_ln[:, m:m + 1], rk[:], MULT, MULT)
                    pending.append(b_norm)
                    if nch + 1 < NCH:
                        hn_cur = hn_next
                # RoPE on kraw [64, S] -> kpe64 (bf16)
                rot = pkr.tile([DR, S], BF16, tag="rotk")
                nc.vector.tensor_copy(rot[0:DR // 2, :], kraw[DR // 2:DR, :])
                nc.vector.tensor_copy(rot[DR // 2:DR, :], kraw[0:DR // 2, :])
                nc.vector.tensor_tensor(kraw[:], kraw[:], cosf[:], MULT)
                nc.vector.tensor_tensor(rot[:], rot[:], sinsf[:], MULT)
                nc.vector.tensor_tensor(kpe64[:], kraw[:], rot[:], ADD)

            # ---- phase C: qT panel (+ RoPE on pe part) ----
            with tc.tile_pool(name="wC", bufs=4) as pcw:
                for m in range(MB_NOPE + MB_PE):
                    wm = pcw.tile([P, KB_QLR, P], BF16, tag="wqb")
                    nc.sync.dma_start(wm[:, :KB_QLR // 2, :],
                                      t["w_qb_t"][m, :, :KB_QLR // 2, :])
                    nc.sync.dma_start(wm[:, KB_QLR // 2:, :],
                                      t["w_qb_t"][m, :, KB_QLR // 2:, :])
                    if pending:
                        pending.pop(0)()
                    ps = psA.tile([P, W], F32, tag="psA")
                    for k in range(KB_QLR):
                        _mm(nc, ps[:], wm[:, k, :], qaT[:, k, :],
                            k == 0, k == KB_QLR - 1)
                    if m < MB_NOPE:
                        nc.scalar.activation(qnopeT[:, m, :], ps[:], COPY)
                    else:
                        j = m - MB_NOPE
                        rotq = prot.tile([P, W], F32, tag="rotq")
                        for h in (0, DR):
                            nc.vector.tensor_copy(rotq[h:h + 32, :],
                                                  ps[h + 32:h + 64, :])
                            nc.vector.tensor_copy(rotq[h + 32:h + 64, :],
                                                  ps[h:h + 32, :])
                        nc.vector.tensor_tensor(rotq[:], rotq[:], sin2sp[:],
                                                MULT)
                        qpe2 = prot.tile([P, W], F32, tag="qpe2")
                        nc.vector.tensor_tensor(qpe2[:], ps[:], cos2p[:],
                                                MULT)
                        nc.vector.tensor_tensor(qpe2[:], qpe2[:], rotq[:],
                                                ADD)
                        nc.vector.tensor_copy(qpe64[:, 2 * j, :],
                                              qpe2[0:DR, :])
                        nc.vector.tensor_copy(qpe64[:, 2 * j + 1, :],
                                              qpe2[DR:P, :])

        # ---------------- phase D: attention per 2-head group -------------
        with tc.tile_pool(name="phD_v", bufs=2) as pdv, \
             tc.tile_pool(name="phD_k", bufs=2) as pdk, \
             tc.tile_pool(name="pracc", bufs=3) as pacc, \
             tc.tile_pool(name="prrow", bufs=3) as prow, \
             tc.tile_pool(name="probs", bufs=4) as pprob, \
             tc.tile_pool(name="psSc", bufs=3, space="PSUM") as psSc, \
             tc.tile_pool(name="psO", bufs=2, space="PSUM") as psO, \
             tc.tile_pool(name="psR", bufs=1, space="PSUM") as psR, \
             tc.tile_pool(name="psPk", bufs=2, space="PSUM") as psPk, \
             ExitStack() as dctx:
            if with_mask:
                mask_pool = dctx.enter_context(
                    tc.tile_pool(name="maskp", bufs=4))

            packA_ps = psPk.tile([NH - 2, W], F32, tag="pk")
            packB_ps = psPk.tile([P, W], F32, tag="pk")

            def head_tail(h, pracc_t, po):
                """denominator reduce, pack into row table, drain po."""
                pr = psR.tile([1, W], F32, tag="pr")
                _mm(nc, pr[:], ones_col[:], pracc_t[:], True, True)
                prow_t = prow.tile([1, W], BF16, tag="prw")
                nc.scalar.activation(prow_t[:], pr[:], COPY)
                if h < NH - 2:
                    _mm(nc, packA_ps[:], packA[:, h, :], prow_t[:],
                        h == 0, h == NH - 3)
                else:
                    _mm(nc, packB_ps[:2, :], packB[:, h - (NH - 2), :],
                        prow_t[:], h == NH - 2, h == NH - 1)
                nc.scalar.activation(oT[:, h, :], po[:], COPY)

            def apply_norm(h):
                """oT[:,h,:] *= broadcast(1/pr_h) via one-hot unpack."""
                bc = psPk.tile([P, W], F32, tag="pk")
                if h < NH - 2:
                    _mm(nc, bc[:], selA[:, h, :], prrA[:], True, True)
                else:
                    _mm(nc, bc[:], selB[:, h - (NH - 2), :], prrB[:],
                        True, True)
                nc.vector.tensor_tensor(oT[:, h, :], oT[:, h, :], bc[:],
                                        MULT)

            for g in range(NH // 2):
                h0, h1 = 2 * g, 2 * g + 1
                # V for the 2 heads: v_sb[k, 2*128]
                if g == 0:
                    wv = wv0
                else:
                    wv = pdw.tile([P, KB_CKV, 2 * DV], BF16, tag="wv")
                    nc.sync.dma_start(wv[:], t["wv_t"][g])
                v_sb = pdv.tile([P, KB_S, 2 * DV], BF16, tag="v")
                for kb in range(KB_S):
                    psv = psSc.tile([P, W], F32, tag="pss")
                    for kc in range(KB_CKV):
                        _mm(nc, psv[:, :2 * DV],
                            ckT[:, kc, kb * P:(kb + 1) * P],
                            wv[:, kc, :], kc == 0, kc == KB_CKV - 1)
                    nc.vector.tensor_copy(v_sb[:, kb, :], psv[:, :2 * DV])

                # knopeT for both heads: [128 d, S] each
                knT = []
                for hl in range(2):
                    h = h0 + hl
                    if g == 0:
                        wkn = wkn0 if hl == 0 else wkn1
                    else:
                        wkn = pdw.tile([P, KB_CKV, DN], BF16, tag="wkn")
                        nc.sync.dma_start(wkn[:], t["wkn_t"][h])
                    kn = pdk.tile([P, KB_S, P], BF16, tag="knT")
                    for nch in range(NCH):
                        psk = psSc.tile([P, W], F32, tag="pss")
                        for kc in range(KB_CKV):
                            _mm(nc, psk[:], wkn[:, kc, :],
                                ckT[:, kc, nch * W:(nch + 1) * W],
                                kc == 0, kc == KB_CKV - 1)
                        nc.vector.tensor_copy(
                            kn[:, nch * (W // P):(nch + 1) * (W // P), :],
                            psk[:])
                    knT.append(kn)

                if g == NH // 2 - 1:
                    # prefetch the first o_proj weight tiles under the
                    # last attention group
                    wo_pre = []
                    for m in range(3):
                        wt = pew.tile([P, NH, P], BF16, tag="wo")
                        nc.sync.dma_start(wt[:], t["w_o_t"][m])
                        wo_pre.append(wt)

                # attention: both heads interleaved over key blocks
                po0 = psO.tile([P, W], F32, tag="po")
                po1 = psO.tile([P, W], F32, tag="po")
                pra0 = pacc.tile([P, W], F32R, tag="pra")
                pra1 = pacc.tile([P, W], F32R, tag="pra")
                for kb in range(KB_S):
                    kbs = slice(kb * P, (kb + 1) * P)
                    pss0 = psSc.tile([P, W], F32, tag="pss")
                    pss1 = psSc.tile([P, W], F32, tag="pss")
                    _mm(nc, pss0[:], knT[0][:, kb, :], qnopeT[:, h0, :],
                        True, False)
                    _mm(nc, pss1[:], knT[1][:, kb, :], qnopeT[:, h1, :],
                        True, False)
                    _mm(nc, pss0[:], kpe64[:, kbs], qpe64[:, h0, :],
                        False, True)
                    _mm(nc, pss1[:], kpe64[:, kbs], qpe64[:, h1, :],
                        False, True)
                    for hl, pss, po, pra in ((0, pss0, po0, pra0),
                                             (1, pss1, po1, pra1)):
                        probs = pprob.tile([P, W], BF16, tag="probs")
                        if with_mask:
                            mtile = mask_pool.tile([P, W], F32, tag="mt")
                            nc.sync.dma_start(mtile[:], t["maskT"][kbs, :])
                            psc = pacc.tile([P, W], F32R, tag="msc")
                            nc.vector.scalar_tensor_tensor(
                                psc[:], pss[:], SCALE, mtile[:], MULT, ADD)
                            nc.scalar.activation(probs[:], psc[:], EXP)
                        else:
                            nc.scalar.activation(probs[:], pss[:], EXP,
                                                 scale=SCALE)
                        _mm(nc, po[:], v_sb[:, kb, hl * DV:(hl + 1) * DV],
                            probs[:], kb == 0, kb == KB_S - 1)
                        if kb == 0:
                            nc.vector.tensor_copy(pra[:], probs[:])
                        else:
                            nc.vector.tensor_tensor(pra[:], pra[:],
                                                    probs[:], ADD)
                head_tail(h0, pra0, po0)
                head_tail(h1, pra1, po1)

                if h1 == NH - 3:
                    # heads 0..13 packed: their batched reciprocal runs
                    # under the last group's attention.
                    nc.scalar.activation(prallA[:], packA_ps[:], COPY)
                    with nc.allow_low_precision(reason="f32r is f32 storage"):
                        nc.vector.reciprocal(prrA[:], prallA[:])

            nc.scalar.activation(prallB[:], packB_ps[:2, :], COPY)
            with nc.allow_low_precision(reason="f32r is f32 storage"):
                nc.vector.reciprocal(prrB[:], prallB[:])
            for h in range(NH):
                apply_norm(h)

        # ---------------- phase E: o_proj ---------------------------------
        with tc.tile_pool(name="phE", bufs=2) as pe, \
             tc.tile_pool(name="psE", bufs=2, space="PSUM") as psE:
            for m in range(MB_HID):
                if m < 3:
                    wm = wo_pre[m]
                else:
                    wm = pew.tile([P, NH, P], BF16, tag="wo")
                    nc.sync.dma_start(wm[:], t["w_o_t"][m])
                ps = psE.tile([P, W], F32, tag="psE")
                for k in range(NH):
                    _mm(nc, ps[:], wm[:, k, :], oT[:, k, :],
                        k == 0, k == NH - 1)
                osb = pe.tile([P, W], F32, tag="osb")
                nc.scalar.activation(osb[:], ps[:], COPY)
                nc.sync.dma_start(t["outT"][:, m, :], osb[:])


def _build_program(with_mask):
    nc = bacc.Bacc("TRN2", target_bir_lowering=False, debug=False)
    t = {}

    def inp(name, shape, dt=F32):
        t[name] = nc.dram_tensor(name, list(shape), dt,
                                 kind="ExternalInput").ap()

    inp("hsT_t", [P, KB_HID, S], BF16)
    inp("hs_panel_t", [P, KB_HID, W], BF16)
    inp("w_qa_t", [MB_QLR, P, KB_HID, P], BF16)
    inp("w_qb_t", [MB_NOPE + MB_PE, P, KB_QLR, P], BF16)
    inp("w_kva_t", [MB_KVA, P, KB_HID, P], BF16)
    inp("wv_t", [NH // 2, P, KB_CKV, 2 * DV], BF16)
    inp("wkn_t", [NH, P, KB_CKV, DN], BF16)
    inp("w_o_t", [MB_HID, P, NH, P], BF16)
    inp("qa_ln_p", [P, KB_QLR])
    inp("kva_ln_p", [P, KB_CKV])
    inp("cos2p", [P, W], BF16)
    inp("sin2sp", [P, W], BF16)
    inp("cosf64", [DR, S], BF16)
    inp("sinsf64", [DR, S], BF16)
    inp("selA", [NH - 2, NH - 2, P], BF16)
    inp("selB", [2, 2, P], BF16)
    inp("packA", [1, NH - 2, NH - 2], BF16)
    inp("packB", [1, 2, 2], BF16)
    if with_mask:
        inp("maskT", [S, W])
    t["outT"] = nc.dram_tensor("outT", [P, MB_HID, W], F32,
                               kind="ExternalOutput").ap()

    with tile.TileContext(nc) as tc:
        _emit(tc, t, with_mask)
    nc.compile()
    return nc


_PROG_CACHE = {}


def _get_program(with_mask):
    if with_mask not in _PROG_CACHE:
        _PROG_CACHE[with_mask] = _build_program(with_mask)
    return _PROG_CACHE[with_mask]


def _tile_w(w, colw=P):
    """[R, C] -> [C//colw, 128, R//128, colw] bf16 contiguous."""
    R, C = w.shape
    kb = R // P
    mb = C // colw
    # wt[m, p, k, c] = w[k*128+p, m*colw+c]
    wt = w.reshape(kb, P, mb, colw).transpose(2, 1, 0, 3)
    return np.ascontiguousarray(wt.astype(ml_dtypes.bfloat16))


def make_in_maps(hidden_states, attention_mask, cos, sin, w_qa, qa_ln, w_qb,
                 w_kva, kva_ln, w_kvb, w_o, with_mask):
    f32 = np.float32
    c = np.ascontiguousarray

    w_qb_r = np.asarray(w_qb).reshape(QLR, NH, DQK)
    w_qb_re = np.concatenate(
        [w_qb_r[:, :, :DN].reshape(QLR, NH * DN),
         w_qb_r[:, :, DN:].reshape(QLR, NH * DR)], axis=1).astype(f32)
    w_kvb_r = np.asarray(w_kvb).reshape(KVLR, NH, DN + DV)
    w_kn = w_kvb_r[:, :, :DN].reshape(KVLR, NH * DN).astype(f32)
    w_v = w_kvb_r[:, :, DN:].reshape(KVLR, NH * DV).astype(f32)
    w_kva_pad = np.zeros((HID, MB_KVA * P), f32)
    w_kva_pad[:, :KVLR + DR] = np.asarray(w_kva).astype(f32)

    qa_ln_p = c(np.asarray(qa_ln).reshape(KB_QLR, P).T.astype(f32))
    kva_ln_p = c(np.asarray(kva_ln).reshape(KB_CKV, P).T.astype(f32))

    cosT = np.asarray(cos).T.astype(f32)                  # [64, S]
    sinT = np.asarray(sin).T.astype(f32)
    sin_s = np.concatenate([-sinT[:DR // 2], sinT[DR // 2:]], axis=0)
    cos2 = np.concatenate([cosT, cosT], axis=0)           # [128, S]
    sin2s = np.concatenate([sin_s, sin_s], axis=0)

    shared = {
        "w_qa_t": _tile_w(np.asarray(w_qa).astype(f32)),
        "w_qb_t": _tile_w(w_qb_re),
        "w_kva_t": _tile_w(w_kva_pad),
        "wv_t": _tile_w(w_v, colw=2 * DV),
        "wkn_t": _tile_w(w_kn, colw=DN),
        "w_o_t": _tile_w(np.asarray(w_o).astype(f32)),
        "qa_ln_p": qa_ln_p,
        "kva_ln_p": kva_ln_p,
        "cosf64": c(cosT.astype(ml_dtypes.bfloat16)),
        "sinsf64": c(sin_s.astype(ml_dtypes.bfloat16)),
        "selA": c((np.eye(NH - 2, dtype=f32)[:, :, None]
                   * np.ones((1, 1, P), f32)).astype(ml_dtypes.bfloat16)),
        "selB": c((np.eye(2, dtype=f32)[:, :, None]
                   * np.ones((1, 1, P), f32)).astype(ml_dtypes.bfloat16)),
        "packA": c(np.eye(NH - 2, dtype=f32)[None].astype(ml_dtypes.bfloat16)),
        "packB": c(np.eye(2, dtype=f32)[None].astype(ml_dtypes.bfloat16)),
    }

    hs = np.asarray(hidden_states)
    am = np.asarray(attention_mask)
    in_maps = []
    for core in range(NCORES):
        b, pnl = divmod(core, NPANEL)
        q0 = pnl * W
        # hsT_t[p, k, s] = hs[b, s, k*128+p]
        hsT_t = np.asarray(hs[b]).astype(ml_dtypes.bfloat16) \
            .reshape(S, KB_HID, P).transpose(2, 1, 0)
        m = dict(shared)
        m["hsT_t"] = c(hsT_t)
        m["hs_panel_t"] = c(hsT_t[:, :, q0:q0 + W])
        m["cos2p"] = c(cos2[:, q0:q0 + W].astype(ml_dtypes.bfloat16))
        m["sin2sp"] = c(sin2s[:, q0:q0 + W].astype(ml_dtypes.bfloat16))
        if with_mask:
            m["maskT"] = c(am[b, 0, q0:q0 + W, :].T.astype(f32))
        in_maps.append(m)
    return in_maps


def kernel(hidden_states, attention_mask, cos, sin, w_qa, qa_ln, w_qb,
           w_kva, kva_ln, w_kvb, w_o):
    global LAST_RESULT
    with_mask = bool(np.any(np.asarray(attention_mask) != 0))
    nc = _get_program(with_mask)
    in_maps = make_in_maps(hidden_states, attention_mask, cos, sin, w_qa,
                           qa_ln, w_qb, w_kva, kva_ln, w_kvb, w_o, with_mask)
    trace = os.environ.get("KERNEL_TRACE", "0") == "1"
    res = bass_utils.run_bass_kernel_spmd(
        nc, in_maps, core_ids=list(range(NCORES)), trace=trace)
    LAST_RESULT = res

    out = np.empty((B, S, HID), np.float32)
    for core in range(NCORES):
        b, pnl = divmod(core, NPANEL)
        q0 = pnl * W
        # outT[p, m, s] -> out[b, q0+s, m*128+p]
        out[b, q0:q0 + W, :] = res.results[core]["outT"] \
            .transpose(2, 1, 0).reshape(W, HID)
    return out
